# revision 29
# baseline (speedup 1.0000x reference)
"""Trainium2 Bass kernel for nn_BaseModel_63058709840114 (dense_mlp).

Reference model per row (d_in=10, d=12):
    h  = x @ We + be
    n1 = LN(h) * g1 + bn1
    m  = relu(n1 @ W1 + b1) @ W2 + b2
    h2 = h + m
    out = (LN(h2) * gh + bnh) @ Wh + bh

Strategy (pure data parallel over 8 cores, batch sharded):
  * All weights folded on host with the centering projection C = I - J/12:
      hc  = x @ A1 + c1          (A1 = We@C, c1 = be@C)  -- centered h
      z   = (hc @ W1g) * r1 + b1f                         (W1g = diag(g1)W1)
      a   = relu(z)
      h2c = hc + a @ W2C + c2    (centered h2; c12 = c1 + c2)
      out = ((h2c) * r2) @ Whg + bhf
    r = 1/sqrt(mean(*c^2) + eps) per row.
  * On-chip layout: feature-major "blocked" tiles. A supertile is 5120 rows =
    128 partitions x 10 rows(t) x 4 chunks(c). PE transposes [128,100] natural
    slices into [100,128] feature-major (10 blocks x 10 feats on partitions,
    batch on free dim, N=512). Hidden tensors are [120,512] (10 blocks x 12).
  * Matmuls are blocked-diagonal bf16. Row-variances come from a matmul whose
    lhsT replicates each block's mean-square to all 12 partitions of the block
    (so no separate broadcast step); rsqrt = ACT sqrt + DVE
    reciprocal_approx_fast, consumed directly as fp32.
  * Emission is phase-interleaved across the 4 supertiles of each DMA quad so
    every engine has independent work from neighboring supertiles in its
    instruction stream (the streams are emission-ordered).
  * Final matmul uses the activation as the stationary operand so the output
    lands natural-layout [128 batch, 120 feat] fp32 in PSUM, evacuated with a
    fused +bias add and DMA'd out contiguously.
"""

import os
import sys
import numpy as np
import ml_dtypes

sys.path.insert(0, "/opt/trn_rl_repo")

EPS = 1e-5
D_IN, D = 10, 12
G = 10                      # feature blocks per supertile (t-rows per chunk)
NCOL = 512                  # free dim of compute tiles (4 chunks of 128)
ROWS_ST = 128 * G * 4       # rows per supertile = 5120
N_CORES = 8


def _fold_weights(w):
    f64 = {k: np.asarray(v, dtype=np.float64) for k, v in w.items()}
    C = np.eye(D) - np.ones((D, D)) / D
    A1 = f64["w_embed"] @ C
    c1 = f64["b_embed"] @ C
    W1g = np.diag(f64["g_norm1"]) @ f64["w_fc1"]
    b1f = f64["b_norm1"] @ f64["w_fc1"] + f64["b_fc1"]
    W2C = f64["w_fc2"] @ C
    c2 = f64["b_fc2"] @ C
    Whg = np.diag(f64["g_normh"]) @ f64["w_head"]
    bhf = f64["b_normh"] @ f64["w_head"] + f64["b_head"]
    return dict(A1=A1, c1=c1, W1g=W1g, b1f=b1f, W2C=W2C, c12=c1 + c2, Whg=Whg, bhf=bhf)


def _block_diag(M, nblk):
    """[k,m] -> [nblk*k, nblk*m] block diagonal."""
    k, m = M.shape
    out = np.zeros((nblk * k, nblk * m), dtype=M.dtype)
    for t in range(nblk):
        out[t * k:(t + 1) * k, t * m:(t + 1) * m] = M
    return out


def make_consts(w):
    """Host-side constant tensors fed to every core."""
    f = _fold_weights(w)
    bf16 = ml_dtypes.bfloat16
    consts = {}
    consts["a1blk"] = _block_diag(f["A1"].astype(np.float32), G).astype(bf16)          # [100,120]
    consts["w1gblk"] = _block_diag(f["W1g"].astype(np.float32), G).astype(bf16)        # [120,120]
    consts["w2cblk"] = _block_diag(f["W2C"].astype(np.float32), G).astype(bf16)        # [120,120]
    consts["whgblk"] = _block_diag(f["Whg"].astype(np.float32), G).astype(bf16)        # [120,120]
    vrep = np.zeros((120, 120), dtype=np.float32)
    for t in range(G):
        vrep[t * D:(t + 1) * D, t * D:(t + 1) * D] = 1.0 / D
    consts["vrep"] = vrep.astype(bf16)                                                 # [120,120]
    consts["c1v"] = np.tile(f["c1"], G).astype(np.float32).reshape(120, 1)
    consts["b1v"] = np.tile(f["b1f"], G).astype(np.float32).reshape(120, 1)
    consts["c2v"] = np.tile(f["c12"] - f["c1"], G).astype(np.float32).reshape(120, 1)  # c2 only
    consts["bhnat"] = np.tile(f["bhf"].astype(np.float32), 40).reshape(1, 480).repeat(128, 0).copy()  # [128,480]
    consts["ident"] = np.eye(128, dtype=np.float32).astype(bf16)                       # [128,128]
    consts["epsv"] = np.full((128, 1), EPS, dtype=np.float32)
    return consts


CONST_SPECS = [
    # name, shape, dtype ("bf16"/"f32")
    ("a1blk", (100, 120), "bf16"),
    ("w1gblk", (120, 120), "bf16"),
    ("w2cblk", (120, 120), "bf16"),
    ("whgblk", (120, 120), "bf16"),
    ("vrep", (120, 120), "bf16"),
    ("c1v", (120, 1), "f32"),
    ("b1v", (120, 1), "f32"),
    ("c2v", (120, 1), "f32"),
    ("bhnat", (128, 480), "f32"),
    ("ident", (128, 128), "bf16"),
    ("epsv", (128, 1), "f32"),
]

# If scalar_tensor_tensor with both tensor operands in PSUM fails on HW,
# set False to evacuate R1' to SBUF first.
STT_TWO_PSUM = False

LAST_EXEC_NS = None


def build_nc(b_core):
    """Build the per-core Bass program. b_core rows must be divisible by 128*G*4*4."""
    import concourse.bass as bass
    import concourse.bacc as bacc
    import concourse.mybir as mybir
    import concourse.tile as tile

    dt = mybir.dt
    BF, F32 = dt.bfloat16, dt.float32
    AF = mybir.ActivationFunctionType
    OP = mybir.AluOpType

    assert b_core % (ROWS_ST * 4) == 0
    n_st = b_core // ROWS_ST          # supertiles
    n_quad = n_st // 4

    nc = bacc.Bacc("TRN2", target_bir_lowering=False, debug=False)
    x_d = nc.dram_tensor("x", [b_core, D_IN], F32, kind="ExternalInput")
    out_d = nc.dram_tensor("out", [b_core, D], F32, kind="ExternalOutput")
    cd = {}
    for name, shape, ty in CONST_SPECS:
        cd[name] = nc.dram_tensor(name, list(shape), BF if ty == "bf16" else F32,
                                  kind="ExternalInput")

    # natural-layout views: row = p*(n_st*40) + s*40 + r   (r = c*10 + t)
    xv = x_d.ap().rearrange("(p s r) f -> p s (r f)", p=128, s=n_st, r=40)    # [128,n_st,400]
    ov = out_d.ap().rearrange("(p s r) j -> p s (r j)", p=128, s=n_st, r=40)  # [128,n_st,480]

    with tile.TileContext(nc) as tc:
        with (
            tc.tile_pool(name="const", bufs=1) as constp,
            tc.tile_pool(name="xin", bufs=4) as xinp,
            tc.tile_pool(name="sm", bufs=2, space="PSUM") as smp,
            tc.tile_pool(name="hcp", bufs=2, space="PSUM") as hcpp,
            tc.tile_pool(name="zm", bufs=2, space="PSUM") as zmp,
            tc.tile_pool(name="vp", bufs=2, space="PSUM") as vpp,
            tc.tile_pool(name="xts", bufs=8) as xtsp,
            tc.tile_pool(name="hcs", bufs=14) as hcsp,
            tc.tile_pool(name="sq", bufs=10) as sqp,
            tc.tile_pool(name="zs", bufs=8) as zsp,
            tc.tile_pool(name="as_", bufs=8) as asp,
            tc.tile_pool(name="h2", bufs=14) as h2p,
            tc.tile_pool(name="n2", bufs=8) as n2p,
            tc.tile_pool(name="stats", bufs=10) as statp,
            tc.tile_pool(name="outf", bufs=4) as outfp,
        ):
            # ---- constants into SBUF
            cs = {}
            for name, shape, ty in CONST_SPECS:
                t = constp.tile(list(shape), BF if ty == "bf16" else F32, tag=name)
                nc.sync.dma_start(out=t[:], in_=cd[name].ap())
                cs[name] = t

            for g in range(n_quad):
                xin = xinp.tile([128, 1600], BF)
                nc.gpsimd.dma_start(
                    out=xin[:].rearrange("p (s r) -> p s r", s=4),
                    in_=xv[:, 4 * g:4 * g + 4, :],
                )  # fp32->bf16 cast
                outf = outfp.tile([128, 4 * 480], F32)
                hcs_k, r1_k, h2_k, r2_k = [], [], [], []
                # ---- phase 1: transpose, embed, variance 1 (all 4 supertiles)
                for k in range(4):
                    xtp = smp.tile([100, NCOL], BF, tag="sm")
                    for c in range(4):
                        nc.tensor.transpose(
                            xtp[:, 128 * c:128 * (c + 1)],
                            xin[:, k * 400 + 100 * c:k * 400 + 100 * (c + 1)],
                            cs["ident"][:],
                        )
                    xts = xtsp.tile([100, NCOL], BF)
                    nc.scalar.copy(xts[:], xtp[:])
                    hcp = hcpp.tile([120, NCOL], F32)
                    nc.tensor.matmul(hcp[:], cs["a1blk"][:], xts[:], start=True, stop=True)
                    hcs = hcsp.tile([120, NCOL], BF)
                    nc.scalar.activation(hcs[:], hcp[:], AF.Identity, bias=cs["c1v"][:, 0:1])
                    sq = sqp.tile([120, NCOL], BF)
                    nc.gpsimd.tensor_mul(sq[:], hcs[:], hcs[:])
                    v1p = vpp.tile([120, NCOL], F32, tag="vp")
                    nc.tensor.matmul(v1p[:], cs["vrep"][:], sq[:], start=True, stop=True)
                    s1 = statp.tile([120, NCOL], F32, tag="s")
                    nc.scalar.activation(s1[:], v1p[:], AF.Sqrt, bias=cs["epsv"][0:120, 0:1])
                    r1 = statp.tile([120, NCOL], F32, tag="r")
                    nc.vector.reciprocal_approx_fast(r1[:], s1[:])
                    hcs_k.append(hcs)
                    r1_k.append(r1)
                # ---- phase 2: mlp + variance 2
                for k in range(4):
                    zp = zmp.tile([120, NCOL], F32, tag="zm")
                    nc.tensor.matmul(zp[:], cs["w1gblk"][:], hcs_k[k][:], start=True, stop=True)
                    zs = zsp.tile([120, NCOL], BF)
                    nc.vector.scalar_tensor_tensor(
                        zs[:], zp[:], 1.0, r1_k[k][:], OP.mult, OP.mult)
                    a_s = asp.tile([120, NCOL], BF)
                    nc.scalar.activation(a_s[:], zs[:], AF.Relu, bias=cs["b1v"][:, 0:1])
                    mp = zmp.tile([120, NCOL], F32, tag="zm")
                    nc.tensor.matmul(mp[:], cs["w2cblk"][:], a_s[:], start=True, stop=True)
                    h2 = h2p.tile([120, NCOL], BF)
                    nc.vector.scalar_tensor_tensor(
                        h2[:], mp[:], cs["c2v"][:, 0:1], hcs_k[k][:], OP.add, OP.add)
                    sq2 = sqp.tile([120, NCOL], BF)
                    nc.gpsimd.tensor_mul(sq2[:], h2[:], h2[:])
                    v2p = vpp.tile([120, NCOL], F32, tag="vp")
                    nc.tensor.matmul(v2p[:], cs["vrep"][:], sq2[:], start=True, stop=True)
                    s2 = statp.tile([120, NCOL], F32, tag="s")
                    nc.scalar.activation(s2[:], v2p[:], AF.Sqrt, bias=cs["epsv"][0:120, 0:1])
                    r2 = statp.tile([120, NCOL], F32, tag="r")
                    nc.vector.reciprocal_approx_fast(r2[:], s2[:])
                    h2_k.append(h2)
                    r2_k.append(r2)
                # ---- phase 3: head + output
                for k in range(4):
                    n2 = n2p.tile([120, NCOL], BF)
                    nc.vector.tensor_mul(n2[:], r2_k[k][:], h2_k[k][:])
                    op_ = smp.tile([128, 480], F32, tag="sm")
                    for c in range(4):
                        nc.tensor.matmul(
                            op_[:, 120 * c:120 * (c + 1)],
                            n2[:, 128 * c:128 * (c + 1)],
                            cs["whgblk"][:],
                            start=True, stop=True, skip_group_check=True,
                        )
                    nc.vector.tensor_add(outf[:, 480 * k:480 * (k + 1)], op_[:], cs["bhnat"][:])
                s4 = 4 * g
                nc.sync.dma_start(
                    out=ov[:, s4:s4 + 4, :],
                    in_=outf[:].rearrange("p (s r) -> p s r", s=4),
                )
    nc.compile()
    return nc


def _shard_and_pad(x, b_core):
    B = x.shape[0]
    per = B // N_CORES
    shards = []
    for i in range(N_CORES):
        s = x[i * per:(i + 1) * per]
        if b_core > per:
            s = np.concatenate([s, np.zeros((b_core - per, x.shape[1]), x.dtype)])
        shards.append(np.ascontiguousarray(s))
    return shards, per


def kernel(**inputs):
    x = np.asarray(inputs["x"], dtype=np.float32)
    B = x.shape[0]
    per = B // N_CORES                      # 524288
    unit = ROWS_ST * 4                      # 20480 (quad)
    b_core = ((per + unit - 1) // unit) * unit   # 532480
    consts = make_consts({k: np.asarray(v) for k, v in inputs.items() if k != "x"})

    nc = build_nc(b_core)
    shards, per = _shard_and_pad(x, b_core)
    in_maps = []
    for i in range(N_CORES):
        m = {"x": shards[i]}
        for name, shape, ty in CONST_SPECS:
            m[name] = np.ascontiguousarray(
                consts[name].astype(ml_dtypes.bfloat16 if ty == "bf16" else np.float32))
        in_maps.append(m)

    results, exec_ns = _run_pjrt(nc, in_maps)
    global LAST_EXEC_NS
    LAST_EXEC_NS = exec_ns
    out = np.concatenate([r[:per] for r in results], axis=0)
    return out.astype(np.float32)


_NC_CACHE = {}


def _run_pjrt(nc, in_maps):
    """Run the bass program on 8 cores via PJRT (axon) and time a second
    steady-state execution with inputs already on device."""
    import time
    import jax
    import concourse.mybir as mybir
    from jax.sharding import Mesh, PartitionSpec
    from jax.experimental.shard_map import shard_map
    from concourse.bass2jax import (
        install_neuronx_cc_hook, _bass_exec_p, partition_id_tensor)

    install_neuronx_cc_hook()
    n_cores = len(in_maps)
    partition_name = nc.partition_id_tensor.name if nc.partition_id_tensor else None

    in_names, out_names, out_avals, zero_outs = [], [], [], []
    for alloc in nc.m.functions[0].allocations:
        if not isinstance(alloc, mybir.MemoryLocationSet):
            continue
        name = alloc.memorylocations[0].name
        if alloc.kind == "ExternalInput":
            if name != partition_name:
                in_names.append(name)
        elif alloc.kind == "ExternalOutput":
            shape = tuple(alloc.tensor_shape)
            dtype = mybir.dt.np(alloc.dtype)
            out_names.append(name)
            out_avals.append(jax.core.ShapedArray(shape, dtype))
            zero_outs.append(np.zeros(shape, dtype))
    n_params = len(in_names)
    n_outs = len(out_avals)
    all_names = in_names + out_names
    if partition_name is not None:
        all_names.append(partition_name)
    donate = tuple(range(n_params, n_params + n_outs))

    def _body(*args):
        operands = list(args)
        if partition_name is not None:
            operands.append(partition_id_tensor())
        outs = _bass_exec_p.bind(
            *operands,
            out_avals=tuple(out_avals),
            in_names=tuple(all_names),
            out_names=tuple(out_names),
            lowering_input_output_aliases=(),
            sim_require_finite=True,
            sim_require_nnan=True,
            nc=nc,
        )
        return tuple(outs)

    devices = jax.devices()[:n_cores]
    mesh = Mesh(np.asarray(devices), ("core",))
    sharded = jax.jit(
        shard_map(_body, mesh=mesh,
                  in_specs=(PartitionSpec("core"),) * (n_params + n_outs),
                  out_specs=(PartitionSpec("core"),) * n_outs,
                  check_rep=False),
        donate_argnums=donate, keep_unused=True,
    )
    concat_in = [
        np.concatenate([np.asarray(in_maps[c][nm]) for c in range(n_cores)], axis=0)
        for nm in in_names
    ]
    concat_zeros = [np.zeros((n_cores * z.shape[0], *z.shape[1:]), z.dtype)
                    for z in zero_outs]

    sh = jax.sharding.NamedSharding(mesh, PartitionSpec("core"))
    dev_in = [jax.device_put(a, sh) for a in concat_in]
    out_arrs = jax.block_until_ready(
        sharded(*dev_in, *[jax.device_put(z, sh) for z in concat_zeros]))
    res_np = [np.asarray(o) for o in out_arrs]

    # Timing: axon dispatch latency (~40-80ms) swamps a single call, so use a
    # non-donating executable (outputs fully written by the kernel, so the
    # zero "out" operands can be reused), dispatch N calls asynchronously and
    # block once; the slope over N is the device execution time.
    exec_ns = None
    if int(os.environ.get("KERNEL_TIME", "0")):
        try:
            fn2 = jax.jit(
                shard_map(_body, mesh=mesh,
                          in_specs=(PartitionSpec("core"),) * (n_params + n_outs),
                          out_specs=(PartitionSpec("core"),) * n_outs,
                          check_rep=False),
                keep_unused=True)
            zs_dev = [jax.device_put(z, sh) for z in concat_zeros]
            jax.block_until_ready(fn2(*dev_in, *zs_dev))  # warm
            times = {}
            for n in (4, 20):
                best = None
                for _ in range(3):
                    t0 = time.perf_counter()
                    outs_l = [fn2(*dev_in, *zs_dev) for _ in range(n)]
                    jax.block_until_ready(outs_l)
                    dt_ = time.perf_counter() - t0
                    best = dt_ if best is None else min(best, dt_)
                    del outs_l
                times[n] = best
            print(f"async batch times: {times}")
            exec_ns = int((times[20] - times[4]) / 16 * 1e9)
        except Exception as e:
            print(f"timing failed: {e}")

    outs = res_np[out_names.index("out")].reshape(n_cores, -1, 12)
    return [outs[c] for c in range(n_cores)], exec_ns


if __name__ == "__main__":
    # quick small-scale numeric check through the simulator
    import concourse.mybir as mybir  # noqa
    from concourse.bass_interp import CoreSim

    rng = np.random.default_rng(0)
    b_core = ROWS_ST * 4  # one quad
    w = {
        "w_embed": rng.uniform(-0.3, 0.3, (D_IN, D)).astype(np.float32),
        "b_embed": rng.uniform(-0.3, 0.3, (D,)).astype(np.float32),
        "g_norm1": np.ones(D, np.float32), "b_norm1": np.zeros(D, np.float32),
        "w_fc1": rng.uniform(-0.3, 0.3, (D, D)).astype(np.float32),
        "b_fc1": rng.uniform(-0.3, 0.3, (D,)).astype(np.float32),
        "w_fc2": rng.uniform(-0.3, 0.3, (D, D)).astype(np.float32),
        "b_fc2": rng.uniform(-0.3, 0.3, (D,)).astype(np.float32),
        "g_normh": np.ones(D, np.float32), "b_normh": np.zeros(D, np.float32),
        "w_head": rng.uniform(-0.3, 0.3, (D, D)).astype(np.float32),
        "b_head": rng.uniform(-0.3, 0.3, (D,)).astype(np.float32),
    }
    x = rng.standard_normal((b_core, D_IN)).astype(np.float32)
    consts = make_consts(w)

    nc = build_nc(b_core)
    sim = CoreSim(nc, trace=False)
    sim.tensor("x")[:] = x
    for name, shape, ty in CONST_SPECS:
        sim.tensor(name)[:] = consts[name].astype(
            ml_dtypes.bfloat16 if ty == "bf16" else np.float32)
    sim.simulate(check_with_hw=False)
    got = np.asarray(sim.tensor("out"))

    from fold_check import reference_np
    ref = reference_np(x.astype(np.float64), {k: v.astype(np.float64) for k, v in w.items()})
    rel = np.linalg.norm(got - ref) / np.linalg.norm(ref)
    mx = np.abs(got - ref).max() / np.abs(ref).max()
    print(f"SIM rel_l2={rel:.3e}  scaled_absmax={mx:.3e}")
    assert rel < 2e-2, "simulation mismatch"
    print("SIM OK")
